# revision 77
# baseline (speedup 1.0000x reference)
"""Trainium2 Bass kernel for nn_H_MAx_C_MaxAtt (pooling attention module).

Reference computation (per sample n):
    x_h[c,h]  = mean_w x + max_w x
    y[m,h]    = conv1_w @ x_h + b ; BN ; h_swish
    a_h[c,h]  = sigmoid(conv_h_w @ y + conv_h_b)
    g[c]      = mean_hw x + max_hw x
    ca[c]     = sigmoid(fc_w @ g + fc_b)
    out       = x * a_h[:, :, None] * ca[:, None, None]

Strategy: data-parallel over batch N (16 samples / 8 cores = 2 per
core), fp16 on the wire both ways (tolerance 2e-2 >> fp16 roundtrip
error), which halves HBM traffic: 16 MiB in + 16 MiB out per core
= ~93 us at the 360 B/ns DMA roofline.

Per core: 2 samples x 16 tiles of [128c, 16h, 128w] fp16.
  - PE  : conv1-mean term computed straight from x tiles via
          shifted-window zero-padded stationary weights (each piece g's
          8 outputs land at PSUM partitions [g*8, g*8+8)), so the
          w-reduction runs 16-wide on DVE; also the max-branch conv1
          (same windows, unscaled), block-diagonal conv_h, and fc.
  - ACT : per-channel global sums via the activation accumulator
          (one Copy+accum pass per tile), plus all sigmoids/BN.
  - DVE : width-max in-place fold tree (TT max at 2x) + small reduce,
          packed-PSUM sum reduces, the small attention chain, and the
          big multiply as one tensor_tensor per tile in 2x_1p mode
          (pair-duplicated scale).
"""

import sys

if "/opt/trn_rl_repo" not in sys.path:
    sys.path.insert(0, "/opt/trn_rl_repo")

from contextlib import ExitStack

import numpy as np

import concourse.bass as bass
import concourse.bacc as bacc
import concourse.tile as tile
from concourse import mybir
from concourse.bass_utils import run_bass_kernel_spmd

F32 = mybir.dt.float32
F16 = mybir.dt.float16
AF = mybir.ActivationFunctionType
ALU = mybir.AluOpType
AX = mybir.AxisListType

N, C, H, W = 16, 256, 128, 128
MIP = 8
N_CORES = 8
NPC = N // N_CORES   # samples per core
CH = C // 128        # channel chunks of 128
HH = 8               # h chunks per sample
HT = H // HH         # h rows per tile (16)
G = 16               # packed pieces per PSUM pack (partition groups of 8)
JT = 4               # h rows per packed piece
WIN = 128 - MIP      # zero margin on each side of the weight window
DVE_ACC = 0          # leading h-chunks whose global-sum accum runs on DVE

EPS = 1e-5


def _build_program(repeats: int = 1) -> bass.Bass:
    nc = bacc.Bacc("TRN2", target_bir_lowering=False, debug=False)

    xd = nc.dram_tensor("x", [NPC, C, H, W], F16, kind="ExternalInput").ap()
    # shifted-window conv1 weights: [zeros(WIN) | w1 | zeros(WIN)]; the
    # slice [WIN - g*8, WIN - g*8 + 128) places piece g's 8 outputs at
    # PSUM partitions [g*8, g*8+8) with zeros elsewhere.  "a" carries the
    # 1/W mean scale, "m" is unscaled for the max branch.
    w1wm_d = nc.dram_tensor("w1wm", [CH, 128, 256], F16,
                            kind="ExternalInput").ap()
    whtz_d = nc.dram_tensor("whtz", [CH, 128, G * 128], F16,
                            kind="ExternalInput").ap()
    fct_d = nc.dram_tensor("fct", [CH, 128, C], F16, kind="ExternalInput").ap()
    # packed biases: [chb0, chb1, fcb0, fcb1, bnsr, bnbr, pad, pad]
    bias_d = nc.dram_tensor("bias", [128, 8], F32, kind="ExternalInput").ap()
    outd = nc.dram_tensor("out", [NPC, C, H, W], F16, kind="ExternalOutput").ap()

    with tile.TileContext(nc) as tc, ExitStack() as ctx:
        consts = ctx.enter_context(tc.tile_pool(name="consts", bufs=1))
        xt_pool = ctx.enter_context(tc.tile_pool(name="xt", bufs=NPC * CH * HH))
        tm_pool = ctx.enter_context(tc.tile_pool(name="tm", bufs=6))
        scr_pool = ctx.enter_context(tc.tile_pool(name="scr", bufs=3))
        small = ctx.enter_context(tc.tile_pool(name="small", bufs=4))
        psum = ctx.enter_context(tc.tile_pool(name="psum", bufs=4, space="PSUM"))
        psum2 = ctx.enter_context(tc.tile_pool(name="psum2", bufs=3, space="PSUM"))

        w1wm_sb = consts.tile([128, CH, 256], F16, tag="w1wm")
        whtz_sb = consts.tile([128, CH, G, 128], F16, tag="whtz")
        fct_sb = consts.tile([128, CH, C], F16, tag="fct")
        bias_sb = consts.tile([128, 8], F32, tag="bias")

        samples = [s for _ in range(repeats) for s in range(NPC)]

        def w1a(ch, g):
            return w1wm_sb[:, ch, WIN - g * MIP: WIN - g * MIP + 128]

        def w1m(ch, g):
            return w1wm_sb[:, ch, WIN - g * MIP: WIN - g * MIP + 128]

        # ---- all loads first (SP queue: no store blocks a later load) ----
        xts = {}
        for si, s in enumerate(samples):
            for hh in range(HH):
                for ch in range(CH):
                    xt = xt_pool.tile([128, HT, W // 2, 2], F16, tag="xt",
                                      name="xt")
                    nc.sync.dma_start(
                        out=xt[:],
                        in_=xd[s, ch * 128:(ch + 1) * 128,
                               hh * HT:(hh + 1) * HT, :],
                    )
                    xts[si, ch, hh] = xt
                if si == 0 and hh == 0:
                    for c2 in range(CH):
                        nc.sync.dma_start(out=w1wm_sb[:, c2, :], in_=w1wm_d[c2])


        for si, s in enumerate(samples):
            # ---- streaming per-tile work ----
            # pack[p][(g, m), r, w] += sum_c w1[m, c]/W * x[c, h, w]
            # for h = p*64 + g*4 + r  (g = (hh % 4) * 4 + j).
            packs = [
                psum.tile([128, JT, W], F32, tag="pack", name=f"pack{si}_{p}")
                for p in range(2)
            ]
            xh_max = [
                small.tile([128, H], F16, tag=f"xhm{ch}", name=f"xhm{si}_{ch}")
                for ch in range(CH)
            ]
            acc8 = [
                small.tile([128, HH], F32, tag=f"acc{ch}",
                           name=f"acc{si}_{ch}")
                for ch in range(CH)
            ]
            for hh in range(HH):
                pk = packs[hh // 4]
                for j in range(JT):
                    g = (hh % 4) * JT + j
                    for ch in range(CH):
                        nc.tensor.matmul(
                            pk[:],
                            lhsT=w1a(ch, g),
                            rhs=xts[si, ch, hh][:, j * 4:(j + 1) * 4, :, :],
                            start=(g == 0 and ch == 0),
                            stop=(g == G - 1 and ch == CH - 1),
                        )
                for ch in range(CH):
                    xt = xts[si, ch, hh]
                    # width-max: in-place fold tree at 2x, then a small reduce
                    tm = tm_pool.tile([128, HT, W // 4, 2], F16, tag="tm",
                                      name="tm")
                    nc.vector.tensor_max(
                        tm[:], xt[:, :, :W // 4, :], xt[:, :, W // 4:, :],
                    )
                    nc.vector.tensor_max(
                        tm[:, :, :W // 8, :], tm[:, :, :W // 8, :],
                        tm[:, :, W // 8:, :],
                    )
                    nc.vector.tensor_max(
                        tm[:, :, :W // 16, :], tm[:, :, :W // 16, :],
                        tm[:, :, W // 16:W // 8, :],
                    )
                    nc.vector.tensor_max(
                        tm[:, :, :W // 32, :], tm[:, :, :W // 32, :],
                        tm[:, :, W // 32:W // 16, :],
                    )
                    nc.vector.reduce_max(
                        xh_max[ch][:, hh * HT:(hh + 1) * HT],
                        tm[:, :, :W // 32, :], axis=AX.XY,
                    )
                    # per-channel global sum via the ACT accumulator
                    scr = scr_pool.tile([128, HT, W // 2, 2], F16,
                                        tag="scr", name="scr")
                    nc.scalar.activation(
                        scr[:], xt[:], AF.Copy,
                        accum_out=acc8[ch][:, hh:hh + 1],
                    )
                if si == 0 and hh == 6:
                    # chain-only parameters; issued from the ACT queue here so
                    # their transfers don't delay the first x tiles
                    for c2 in range(CH):
                        nc.scalar.dma_start(
                            out=whtz_sb[:, c2, :, :].rearrange("p a b -> p (a b)"),
                            in_=whtz_d[c2])
                        nc.scalar.dma_start(out=fct_sb[:, c2, :], in_=fct_d[c2])
                    nc.scalar.dma_start(out=bias_sb[:], in_=bias_d[:])
            # ---- packed sum reduce + max-branch conv1 ----
            sps = psum2.tile([128, 512], F32, tag="sps", name="sps")
            y2s = []
            with nc.allow_low_precision(reason="fp16 intermediates"):
                for p in range(2):
                    red = small.tile([128, JT], F32, tag="red", name="red")
                    nc.vector.reduce_sum(red[:], packs[p][:], axis=AX.X)
                    maxps = sps[:, p * JT:(p + 1) * JT]
                    for g in range(G):
                        h0 = p * 64 + g * JT
                        for ch in range(CH):
                            nc.tensor.matmul(
                                maxps,
                                lhsT=w1m(ch, g),
                                rhs=xh_max[ch][:, h0:h0 + JT],
                                start=(g == 0 and ch == 0),
                                stop=(g == G - 1 and ch == CH - 1),
                            )
                    # y = (mean + max) * bns + bnb ; y2 = y * relu6(y+3)
                    ypre = small.tile([128, JT], F32, tag="ypre", name="ypre")
                    nc.vector.scalar_tensor_tensor(
                        ypre[:], in0=red[:], scalar=1.0 / W, in1=maxps,
                        op0=ALU.mult, op1=ALU.add,
                    )
                    ybn = small.tile([128, JT], F32, tag="ybn", name="ybn")
                    nc.vector.tensor_scalar(
                        out=ybn[:], in0=ypre[:], scalar1=bias_sb[:, 4:5],
                        scalar2=bias_sb[:, 5:6], op0=ALU.mult, op1=ALU.add,
                    )
                    t6 = small.tile([128, JT], F32, tag="t6", name="t6")
                    nc.vector.tensor_scalar(
                        out=t6[:], in0=ybn[:], scalar1=3.0, scalar2=0.0,
                        op0=ALU.add, op1=ALU.max,
                    )
                    nc.vector.tensor_scalar_min(t6[:], t6[:], 6.0)
                    y2 = small.tile([128, JT], F16, tag="y2", name="y2")
                    nc.vector.tensor_mul(y2[:], ybn[:], t6[:])
                    y2s.append(y2)

                # ---- a_h = sigmoid(conv_h/6 @ y2 + chb)  (block-diagonal) ----
                a_ps = sps[:, 16:16 + CH * 128]
                for p in range(2):
                    for g in range(G):
                        h0 = p * 64 + g * JT
                        for ch in range(CH):
                            nc.tensor.matmul(
                                a_ps[:, ch * 128 + h0: ch * 128 + h0 + JT],
                                lhsT=whtz_sb[:, ch, g, :],
                                rhs=y2s[p][:],
                            )

                # ---- ca = sigmoid(fc @ (gsum/HW + gmax) + fcb) ----
                g16 = []
                for ch in range(CH):
                    gmax = small.tile([128, 1], F16, tag="gmax", name="gmax")
                    nc.vector.reduce_max(gmax[:], xh_max[ch][:], axis=AX.X)
                    gsum = small.tile([128, 1], F32, tag="gsum", name="gsum")
                    nc.vector.reduce_sum(gsum[:], acc8[ch][:], axis=AX.X)
                    gt = small.tile([128, 1], F16, tag="g16", name="g16")
                    nc.vector.scalar_tensor_tensor(
                        gt[:], in0=gsum[:], scalar=1.0 / (H * W), in1=gmax[:],
                        op0=ALU.mult, op1=ALU.add,
                    )
                    g16.append(gt)
                ca_ps = sps[:, 280:280 + CH]
                for ch in range(CH):
                    for j in range(CH):
                        nc.tensor.matmul(
                            ca_ps[:, ch:ch + 1],
                            lhsT=fct_sb[:, j, ch * 128:(ch + 1) * 128],
                            rhs=g16[j][:],
                            start=(j == 0), stop=(j == CH - 1),
                        )

                # ---- combine attentions; write pair-duplicated fp16 scale ----
                a2 = small.tile([128, CH, H], F16, tag="a2", name="a2")
                casb = small.tile([128, CH], F32, tag="ca", name="ca")
                a2p = small.tile([128, CH, H, 2], F16, tag="a2p", name="a2p")
                for ch in range(CH):
                    nc.scalar.activation(
                        a2[:, ch, :], a_ps[:, ch * 128:(ch + 1) * 128],
                        AF.Sigmoid, bias=bias_sb[:, ch:ch + 1],
                    )
                    nc.scalar.activation(
                        casb[:, ch:ch + 1], ca_ps[:, ch:ch + 1], AF.Sigmoid,
                        bias=bias_sb[:, 2 + ch:3 + ch],
                    )
                    nc.scalar.activation(
                        a2p[:, ch, :, :],
                        a2[:, ch, :].unsqueeze(2).broadcast_to([128, H, 2]),
                        AF.Copy, scale=casb[:, ch:ch + 1],
                    )

                # ---- out = x * a2 ; store ----
                for hh in range(HH):
                    for ch in range(CH):
                        xt = xts[si, ch, hh]
                        bc = (
                            a2p[:, ch, hh * HT:(hh + 1) * HT, :]
                            .unsqueeze(2)
                            .broadcast_to([128, HT, W // 2, 2])
                        )
                        nc.vector.tensor_mul(xt[:], xt[:], bc)
                        nc.sync.dma_start(
                            out=outd[s, ch * 128:(ch + 1) * 128,
                                     hh * HT:(hh + 1) * HT, :],
                            in_=xt[:],
                        )
    nc.compile()
    return nc


_NC_CACHE = {}


def _get_program(repeats: int = 1) -> bass.Bass:
    if repeats not in _NC_CACHE:
        _NC_CACHE[repeats] = _build_program(repeats)
    return _NC_CACHE[repeats]


def _prep_in_maps(inputs: dict) -> list:
    f = lambda a: np.asarray(a, dtype=np.float32)
    x = np.ascontiguousarray(np.asarray(inputs["x"])).astype(np.float16)
    conv1_w = f(inputs["conv1_w"])
    conv1_b = f(inputs["conv1_b"])
    bn_gamma = f(inputs["bn_gamma"])
    bn_beta = f(inputs["bn_beta"])
    bn_mean = f(inputs["bn_mean"])
    bn_var = f(inputs["bn_var"])
    conv_h_w = f(inputs["conv_h_w"])
    conv_h_b = f(inputs["conv_h_b"])
    fc_w = f(inputs["fc_w"])
    fc_b = f(inputs["fc_b"])

    # Host-side folds:
    #   BN(y) = y*bns + bnb,  bns = gamma/sqrt(var+eps),
    #   conv1 bias applied before BN: bnb = beta + (conv1_b - mean)*bns.
    # bns/bnb replicated 16x across the packed partition groups.
    bns = bn_gamma / np.sqrt(bn_var + EPS)
    bnb = bn_beta + (conv1_b - bn_mean) * bns
    w1t = np.ascontiguousarray(conv1_w.T).reshape(CH, 128, MIP)
    w1wm = np.zeros((CH, 128, 256), np.float32)
    w1wm[:, :, WIN:WIN + MIP] = w1t
    wht6 = np.ascontiguousarray(conv_h_w.T) / 6.0            # [MIP, C]
    # whtz[ch, (g,m), g, c] = wht6[m, ch*128+c]: block-diagonal conv_h that
    # consumes the packed y2 layout directly.
    whtz = np.zeros((CH, 128, G, 128), np.float32)
    for g in range(G):
        for m in range(MIP):
            whtz[:, g * MIP + m, g, :] = wht6[m].reshape(CH, 128)
    fct = np.ascontiguousarray(fc_w.T).reshape(CH, 128, C)   # fct[j, c] = fc_w[c, j]
    shared = {
        "w1wm": w1wm.astype(np.float16),
        "whtz": whtz.reshape(CH, 128, G * 128).astype(np.float16),
        "fct": fct.astype(np.float16),
        "bias": np.stack(
            [conv_h_b[:128], conv_h_b[128:], fc_b[:128], fc_b[128:],
             np.tile(bns, G), np.tile(bnb, G),
             np.zeros(128, np.float32), np.zeros(128, np.float32)],
            axis=1).astype(np.float32),
    }
    return [
        {"x": np.ascontiguousarray(x[i * NPC:(i + 1) * NPC]), **shared}
        for i in range(N_CORES)
    ]


def _run(inputs: dict, trace: bool = False, repeats: int = 1):
    nc = _get_program(repeats)
    in_maps = _prep_in_maps(inputs)
    res = run_bass_kernel_spmd(nc, in_maps, list(range(N_CORES)), trace=trace)
    out = np.concatenate(
        [np.asarray(res.results[i]["out"]) for i in range(N_CORES)], axis=0
    ).astype(np.float32)
    return out, res


def kernel(**inputs) -> np.ndarray:
    out, _ = _run(inputs)
    return out


if __name__ == "__main__":
    rng = np.random.default_rng(0)
    ins = {
        "x": rng.standard_normal((N, C, H, W), dtype=np.float32),
        "conv1_w": rng.standard_normal((MIP, C), dtype=np.float32) * 0.05,
        "conv1_b": rng.standard_normal((MIP,), dtype=np.float32) * 0.05,
        "bn_gamma": np.ones((MIP,), np.float32),
        "bn_beta": np.zeros((MIP,), np.float32),
        "bn_mean": rng.standard_normal((MIP,), dtype=np.float32) * 0.1,
        "bn_var": rng.random((MIP,), dtype=np.float32) * 0.5 + 0.5,
        "conv_h_w": rng.standard_normal((C, MIP), dtype=np.float32) * 0.05,
        "conv_h_b": rng.standard_normal((C,), dtype=np.float32) * 0.05,
        "fc_w": rng.standard_normal((C, C), dtype=np.float32) * 0.05,
        "fc_b": rng.standard_normal((C,), dtype=np.float32) * 0.05,
    }
    out = kernel(**ins)
    print("out", out.shape, out.dtype, float(np.abs(out).max()))
